# revision 1
# baseline (speedup 1.0000x reference)
"""Trainium2 Bass kernel for Luong-style attention.

Reference computation (per full problem):
    h = decoder_hidden @ W.T + b          # [B, De]
    enc = encoder_output.transpose(1,0,2) # [B, S, De]
    a = softmax(einsum('bsd,bd->bs', enc, h), axis=1)
    context = einsum('bs,bsd->bd', a, enc)  # [B, De]

Shapes: B=64, S=4096, Dd=1024, De=512 (f32).

Strategy: data-parallel over B across 8 NeuronCores (B_local=8 each).
encoder_output is the huge tensor (512 MB); each core streams its
64 MB shard from HBM exactly once (chunked two-level softmax):
  - scores in f32 via fused DVE scalar_tensor_tensor (product +
    row-sum in one op) against a partition-broadcast copy of h,
  - each tile also cast f32->bf16 on the Scalar engine for the
    context path (bf16 weights make PE LDWEIGHTS ~5x faster),
  - per-chunk softmax via PE transpose + ACT exp (bias=-chunk_max,
    fused row-sum),
  - context accumulated TRANSPOSED in a single PSUM bank
    (lhsT = bf16 enc d-slice as weights, rhs = prob column, N=1;
    the accumulation group is opened by one bank-wide zero matmul
    because start=True clears has_written bits bank-wide),
  - chunks combined with exp(m_c - M)/l weights at the end,
    partition-broadcasts done with one-hot selector matmuls (no
    cross-partition DMA hops), then transposed back and stored.
No collectives needed.  ~228 us on silicon vs ~179 us single-pass
HBM roofline (64 MB/core at ~358 GB/s); rel err 1.7e-3.
"""

import numpy as np

import concourse.bass as bass
import concourse.bacc as bacc_mod
import concourse.tile as tile
import concourse.mybir as mybir
from concourse import masks
from concourse.bass_utils import run_bass_kernel_spmd

F32 = mybir.dt.float32
BF16 = mybir.dt.bfloat16
ALU = mybir.AluOpType
ACTF = mybir.ActivationFunctionType
AX = mybir.AxisListType

NCORES = 8
B = 8          # per-core batch
S = 4096
DD = 1024
DE = 512
P = 128        # s-values per tile
M = DE // P              # 4 d-chunks
NTILES = S // P          # 32
CHUNK_TILES = 4          # s-tiles per softmax chunk
NCHUNK = NTILES // CHUNK_TILES   # 8


def build_nc(ntiles: int = NTILES):
    nchunk = ntiles // CHUNK_TILES
    s_local = ntiles * P
    nc = bacc_mod.Bacc("TRN2", target_bir_lowering=False, debug=False)
    dec_d = nc.dram_tensor("decoder_hidden", [B, DD], F32, kind="ExternalInput")
    enc_d = nc.dram_tensor("encoder_output", [s_local, B, DE], F32, kind="ExternalInput")
    w_d = nc.dram_tensor("W", [DE, DD], F32, kind="ExternalInput")
    b_d = nc.dram_tensor("b", [DE], F32, kind="ExternalInput")
    out_d = nc.dram_tensor("out", [B, DE], F32, kind="ExternalOutput")

    with tile.TileContext(nc) as tc:
        with (
            tc.tile_pool(name="const", bufs=1) as const_pool,
            tc.tile_pool(name="wload", bufs=4) as wload_pool,
            tc.tile_pool(name="persist", bufs=1) as persist_pool,
            tc.tile_pool(name="enc", bufs=5) as enc_pool,
            tc.tile_pool(name="encbf", bufs=6) as encbf_pool,
            tc.tile_pool(name="scratch", bufs=4) as scratch_pool,
            tc.tile_pool(name="scores", bufs=4) as sc_pool,
            tc.tile_pool(name="probs", bufs=2) as p_pool,
            tc.tile_pool(name="pt", bufs=6) as pt_pool,
        ):
            setup_psum_cm = tc.tile_pool(name="psum_setup", bufs=4, space="PSUM")
            psum_setup = setup_psum_cm.__enter__()
            setup_psum2_cm = tc.tile_pool(name="psum_setup2", bufs=4, space="PSUM")
            psum_setup2 = setup_psum2_cm.__enter__()
            # ---- constants ----
            ident = const_pool.tile([P, P], F32)
            masks.make_identity(nc, ident[:])
            ones = const_pool.tile([1, P], F32)
            nc.vector.memset(ones[:], 1.0)
            zeros_row = const_pool.tile([1, M * B], BF16)
            nc.vector.memset(zeros_row[:], 0.0)
            ones_bf = const_pool.tile([1, P], BF16)
            nc.vector.memset(ones_bf[:], 1.0)
            # row-broadcast selectors: sel[:, bb, :] is [8, 128] with row bb
            # all-ones; matmul(sel_bb, x) broadcasts x's row bb to all
            # 128 partitions without any cross-partition DMA.
            sel = const_pool.tile([B, B, P], F32)
            nc.gpsimd.memset(sel[:], 0.0)
            # sel[k, bb, m] = 1.0 iff k == bb  (k*1 + bb*(-1) == 0)
            nc.gpsimd.affine_select(
                out=sel[:], in_=sel[:],
                compare_op=ALU.not_equal, fill=1.0, base=0,
                pattern=[[-1, B], [0, P]], channel_multiplier=1)

            # ---- load small inputs ----
            dec_sb = const_pool.tile([B, DD], F32)
            nc.sync.dma_start(dec_sb[:], dec_d[:])
            bias_sb = const_pool.tile([1, DE], F32)
            nc.sync.dma_start(bias_sb[:], b_d[None, :])
            setup_dmas = []

            # ---- transpose dec: [8,1024] -> decT [128, 8, 8] (chunk c = cols c*128..) ----
            decT = const_pool.tile([P, DD // P, B], F32)
            for c in range(DD // P):
                tp = psum_setup.tile([P, B], F32, tag="su")
                nc.tensor.transpose(tp[:], dec_sb[:, c * P:(c + 1) * P], ident[0:B, 0:B])
                nc.vector.tensor_copy(decT[:, c, :], tp[:])

            # ---- transpose W: [512,1024] -> WT [128, 8, 512] (chunk c = W.T rows c*128..) ----
            wt_sb = persist_pool.tile([P, DD // P, DE], F32)
            for wi in range(DE // P):
                w_row = wload_pool.tile([P, DD], F32, tag="wrow")
                half = DD // 2
                setup_dmas.append(nc.sync.dma_start(
                    w_row[:, 0:half], w_d[wi * P:(wi + 1) * P, 0:half]))
                setup_dmas.append(nc.sync.dma_start(
                    w_row[:, half:DD], w_d[wi * P:(wi + 1) * P, half:DD]))
                for c in range(DD // P):
                    tp = psum_setup.tile([P, P], F32, tag="su")
                    nc.tensor.transpose(tp[:], w_row[:, c * P:(c + 1) * P], ident[:])
                    nc.vector.tensor_copy(wt_sb[:, c, wi * P:(wi + 1) * P], tp[:])

            # ---- h = dec @ W.T + b  -> h_sb [8, 512] ----
            h_ps = psum_setup2.tile([B, DE], F32, tag="hsu")
            for c in range(DD // P):
                nc.tensor.matmul(h_ps[:], decT[:, c, :], wt_sb[:, c, :],
                                 start=(c == 0), stop=False)
            nc.tensor.matmul(h_ps[:], ones[0:1, 0:B], bias_sb[:],
                             start=False, stop=True)
            h_sb = const_pool.tile([B, DE], F32)
            nc.vector.tensor_copy(h_sb[:], h_ps[:])

            # ---- broadcast h along partitions: hb [128, 8, 512] ----
            # selector matmul: out = sel_bb.T @ h_sb puts h row bb on all
            # 128 partitions; no cross-partition DMA hop in the chain.
            hb = persist_pool.tile([P, B, DE], F32)
            for bb in range(B):
                hp = psum_setup2.tile([P, DE], F32, tag="hsu")
                nc.tensor.matmul(hp[:], sel[:, bb, :], h_sb[:],
                                 start=True, stop=True)
                nc.vector.tensor_copy(hb[:, bb, :], hp[:])

            setup_psum2_cm.__exit__(None, None, None)
            setup_psum_cm.__exit__(None, None, None)
            _tr_cm = tc.tile_pool(name="psum_tr", bufs=4, space="PSUM")
            psum_tr = _tr_cm.__enter__()
            _sc_cm = tc.tile_pool(name="psum_sc", bufs=2, space="PSUM")
            psum_sc = _sc_cm.__enter__()
            _ctx_cm = tc.tile_pool(name="psum_ctx", bufs=2, space="PSUM")
            psum_ctx = _ctx_cm.__enter__()

            # ---- per-chunk stats / outputs ----
            m_all = persist_pool.tile([B, nchunk], F32)
            negm_all = persist_pool.tile([B, nchunk], F32)
            l_all = persist_pool.tile([B, nchunk], F32)
            w_all = persist_pool.tile([B, nchunk], F32)
            # transposed context partials: [de%128, chunk, m, b]
            ctxt_all = persist_pool.tile([P, nchunk, M, B], F32)

            # ---- main streaming loop over S ----
            for c in range(nchunk):
                enc_tiles = []
                scT = psum_sc.tile([B, CHUNK_TILES * P], F32)
                for t in range(CHUNK_TILES):
                    j = c * CHUNK_TILES + t
                    et = enc_pool.tile([P, B, DE], F32)
                    enc_dma = nc.sync.dma_start(et[:], enc_d[j * P:(j + 1) * P, :, :])
                    if j == 0:
                        for sd in setup_dmas:
                            tile.add_dep_helper(enc_dma.ins, sd.ins,
                                                reason="let setup W loads win HBM first")
                    et_bf = encbf_pool.tile([P, B, DE], BF16)
                    nc.scalar.copy(et_bf[:], et[:])
                    enc_tiles.append(et_bf)
                    # scores for this tile: [128, 8]
                    sct = sc_pool.tile([P, B], F32)
                    for bb in range(B):
                        junk = scratch_pool.tile([P, DE], BF16, tag="junk")
                        nc.vector.scalar_tensor_tensor(
                            out=junk[:],
                            in0=et[:, bb, :],
                            scalar=1.0,
                            in1=hb[:, bb, :],
                            op0=ALU.mult,
                            op1=ALU.mult,
                            accum_out=sct[:, bb:bb + 1],
                        )
                    # transpose scores into [8, 128] slice of chunk psum
                    nc.tensor.transpose(scT[:, t * P:(t + 1) * P], sct[:], ident[:])

                # chunk softmax: m_c, p_c, l_c
                nc.vector.reduce_max(m_all[:, c:c + 1], scT[:], axis=AX.X)
                nc.vector.tensor_scalar_mul(negm_all[:, c:c + 1], m_all[:, c:c + 1], -1.0)
                p_sb = p_pool.tile([B, CHUNK_TILES * P], F32)
                nc.scalar.activation(p_sb[:], scT[:], ACTF.Exp,
                                     bias=negm_all[:, c:c + 1], scale=1.0,
                                     accum_out=l_all[:, c:c + 1])

                # transposed context partial, all in ONE psum bank:
                # ctxT[p, m, b] += sum_s enc[s, b, m*128+p] * p_c[s, b]
                # `start=True` clears has_written bits bank-wide, so open the
                # accumulation group once with a bank-covering zero matmul and
                # accumulate everything else with start=False.
                ctx_ps = psum_ctx.tile([P, M, B], F32)
                nc.tensor.matmul(ctx_ps[:], ones_bf[:], zeros_row[:],
                                 start=True, stop=False)
                for t in range(CHUNK_TILES):
                    ptp = psum_tr.tile([P, B], F32, tag="tr")
                    nc.tensor.transpose(ptp[:], p_sb[:, t * P:(t + 1) * P], ident[0:B, 0:B])
                    pts = pt_pool.tile([P, B], BF16)
                    nc.scalar.copy(pts[:], ptp[:])
                    for bb in range(B):
                        for mm in range(M):
                            last = (t == CHUNK_TILES - 1 and bb == B - 1
                                    and mm == M - 1)
                            nc.tensor.matmul(
                                ctx_ps[:, mm, bb:bb + 1],
                                enc_tiles[t][:, bb, mm * P:(mm + 1) * P],
                                pts[:, bb:bb + 1],
                                start=False, stop=last)
                nc.scalar.copy(ctxt_all[:, c, :, :], ctx_ps[:])

            # ---- combine chunks ----
            g_max = persist_pool.tile([B, 1], F32)
            g_negmax = persist_pool.tile([B, 1], F32)
            g_l = persist_pool.tile([B, 1], F32)
            g_rl = persist_pool.tile([B, 1], F32)
            nc.vector.reduce_max(g_max[:], m_all[:], axis=AX.X)
            nc.vector.tensor_scalar_mul(g_negmax[:], g_max[:], -1.0)
            nc.scalar.activation(w_all[:], m_all[:], ACTF.Exp,
                                 bias=g_negmax[:], scale=1.0)
            junk2 = persist_pool.tile([B, nchunk], F32)
            nc.vector.scalar_tensor_tensor(
                out=junk2[:], in0=l_all[:], scalar=1.0, in1=w_all[:],
                op0=ALU.mult, op1=ALU.mult, accum_out=g_l[:])
            nc.vector.reciprocal(g_rl[:], g_l[:])

            # normalized chunk weights: wn[b, c] = w[b, c] / l_total[b]
            w_norm = persist_pool.tile([B, nchunk], F32)
            nc.vector.tensor_scalar(out=w_norm[:], in0=w_all[:],
                                    scalar1=g_rl[:, 0:1], scalar2=None, op0=ALU.mult)
            # broadcast wn along partitions: [128, chunk, b] via selector
            # matmuls (row bb of wn to all partitions, one matmul per b).
            wb = persist_pool.tile([P, nchunk, B], F32)
            for bb in range(B):
                wbp = psum_tr.tile([P, nchunk], F32, tag="tr")
                nc.tensor.matmul(wbp[:], sel[:, bb, :], w_norm[:],
                                 start=True, stop=True)
                nc.scalar.copy(wb[:, :, bb], wbp[:])

            # weighted sum over chunks (still transposed): [128, m, b]
            ctxf = persist_pool.tile([P, M, B], F32)
            for mm in range(M):
                tmp = persist_pool.tile([P, nchunk, B], F32)
                nc.vector.tensor_tensor(out=tmp[:], in0=ctxt_all[:, :, mm, :],
                                        in1=wb[:], op=ALU.mult)
                nc.vector.reduce_sum(
                    ctxf[:, mm, :],
                    tmp[:].rearrange("p c b -> p b c"),
                    axis=AX.X)

            # transpose back to [b, de] and store
            out_sb = persist_pool.tile([B, DE], F32)
            for mm in range(M):
                op_ps = psum_tr.tile([B, P], F32, tag="tr")
                nc.tensor.transpose(op_ps[:], ctxf[:, mm, :], ident[:])
                nc.scalar.copy(out_sb[:, mm * P:(mm + 1) * P], op_ps[:])
            nc.sync.dma_start(out_d[:], out_sb[:])
            _ctx_cm.__exit__(None, None, None)
            _sc_cm.__exit__(None, None, None)
            _tr_cm.__exit__(None, None, None)

    nc.compile()
    if not nc.is_finalized():
        nc.finalize()
    return nc


_NC = None


def kernel(decoder_hidden, encoder_output, W, b):
    global _NC
    if _NC is None:
        _NC = build_nc()
    decoder_hidden = np.ascontiguousarray(decoder_hidden, dtype=np.float32)
    encoder_output = np.ascontiguousarray(encoder_output, dtype=np.float32)
    W = np.ascontiguousarray(W, dtype=np.float32)
    b = np.ascontiguousarray(b, dtype=np.float32)

    in_maps = []
    for i in range(NCORES):
        sl = slice(i * B, (i + 1) * B)
        in_maps.append({
            "decoder_hidden": decoder_hidden[sl],
            "encoder_output": np.ascontiguousarray(encoder_output[:, sl, :]),
            "W": W,
            "b": b,
        })
    res = run_bass_kernel_spmd(_NC, in_maps, core_ids=list(range(NCORES)))
    return np.concatenate([res.results[i]["out"] for i in range(NCORES)], axis=0)

